# revision 16
# baseline (speedup 1.0000x reference)
"""Trainium2 Bass kernel for nn_ExtractNet (multi-task MoE with shared experts).

Contract: kernel(**inputs) takes FULL unsharded numpy inputs (as produced by
setup_inputs) and returns the FULL [B, T, OUT] output. Internally shards the
batch across 8 NeuronCores (data parallel), with all expert/gate weights
replicated.

Math (all biases are zero in this problem):
  out[b,t,:] = sum_e softmax(x_b @ Wg[t])_e * MLP_e(x_b)
with 8 experts per task (4 task-specific + 4 shared), each MLP a zero-bias
relu network 256->64->64->64.

Dataflow (per 512-token tile; features on partitions, tokens on free axis;
bf16 compute, fp32 PSUM):
  - X is transposed + bf16-cast on HOST, so the device does zero transposes
    of activations: L1 consumes the [128, 2, 512] x^T chunks directly.
  - L1 computes 768 expert h1 features + 16 gate logits as 7 M-chunks x
    2 K-chunks (14 matmuls). Gates go first; exp on Scalar; the softmax
    denominator Z comes from one tiny [16,16] block-ones matmul; gates are
    normalized up front (reciprocal + multiply on DVE), so no output
    scaling pass exists at all.
  - Normalized gates roundtrip through DRAM to build a row-broadcast tile
    pball[128, 8, 512] (2 DMAs), feeding fused relu+scale
    scalar_tensor_tensor stacks (split 4 on DVE / 4 on GpSimd).
  - L2 is 6 block-diagonal expert-pair matmuls; L3 accumulates the 8 gated
    stacks per task into one PSUM bank (stacked-K, tile_position column
    packing).
  - The token-major output transpose runs on the DMA XBAR
    (dma_start_transpose), not TensorE. Tiles are stored raw/contiguous in
    bf16; the host does the final unpack + fp32 cast.

A 3-deep software pipeline (tile i: L1/gates, tile i-1: L2/stacks, tile
i-2: L3/out) with L2 matmuls interleaved between L1 M-chunks keeps TensorE
streaming back-to-back: 29 matmuls/tile, all real work.
"""

import os
import sys

for _p in ("/opt/trn_rl_repo", "/root/.axon_site/_ro/trn_rl_repo"):
    if os.path.isdir(_p) and _p not in sys.path:
        sys.path.insert(0, _p)

import numpy as np
import ml_dtypes

B, IN, H, OUT = 65536, 256, 64, 64
T, ET, ES = 2, 4, 4
NCORES = 8
SHARD = B // NCORES  # 8192
TILE = 512
NTILES = SHARD // TILE  # 16

_BUILD_CACHE = {}


def _build(ntiles):
    import concourse.bass as bass
    import concourse.tile as tile
    from concourse import mybir, bacc

    f32, bf16 = mybir.dt.float32, mybir.dt.bfloat16
    Relu = mybir.ActivationFunctionType.Relu
    Exp = mybir.ActivationFunctionType.Exp
    mult = mybir.AluOpType.mult
    amax = mybir.AluOpType.max

    nc = bacc.Bacc()
    XT = nc.declare_dram_parameter("XT", [2, 128, ntiles * TILE], bf16,
                                   isOutput=False)
    W1C = nc.declare_dram_parameter("W1C", [128, 2, 784], bf16, isOutput=False)
    W2B = nc.declare_dram_parameter("W2B", [128, 768], bf16, isOutput=False)
    W3S = nc.declare_dram_parameter("W3S", [128, 512], bf16, isOutput=False)
    O16 = nc.declare_dram_parameter("O16", [16, 16], bf16, isOutput=False)
    OUTP = nc.declare_dram_parameter("out", [ntiles, 128, TILE], bf16,
                                     isOutput=True)
    ntok = ntiles * TILE

    with tile.TileContext(nc) as tc:
        with (
            tc.tile_pool(name="consts", bufs=1) as consts,
            tc.tile_pool(name="sba", bufs=3) as sba,       # xt tiles
            tc.tile_pool(name="sbg", bufs=2) as sbg,       # small gate tiles
            tc.tile_pool(name="sbb", bufs=13) as sbb,      # h1 tiles
            tc.tile_pool(name="sbc", bufs=17) as sbc,      # stacks
            tc.tile_pool(name="sbp", bufs=2) as sbp,       # pball broadcast
            tc.tile_pool(name="sbo", bufs=2) as sbo,       # out staging
            tc.tile_pool(name="drp", bufs=3, space="DRAM") as drp,
            tc.tile_pool(name="psA", bufs=3, space="PSUM") as psA,
            tc.tile_pool(name="psB", bufs=4, space="PSUM") as psB,
            tc.tile_pool(name="psC", bufs=1, space="PSUM") as psC,
        ):
            w1sb = consts.tile([128, 2, 784], bf16)
            nc.sync.dma_start(out=w1sb[:], in_=W1C[:])
            w2sb = consts.tile([128, 768], bf16)
            nc.sync.dma_start(out=w2sb[:], in_=W2B[:])
            w3sb = consts.tile([128, 512], bf16)
            nc.sync.dma_start(out=w3sb[:], in_=W3S[:])
            o16sb = consts.tile([16, 16], bf16)
            nc.sync.dma_start(out=o16sb[:], in_=O16[:])

            def load_xt(it):
                tok0 = it * TILE
                xt = sba.tile([128, 2, TILE], bf16, tag="xt")
                src = XT[:, :, tok0:tok0 + TILE].rearrange("c p t -> p c t")
                nc.gpsimd.dma_start(out=xt[:], in_=src)
                return xt

            def gates_head(xt):
                """Gate logits -> exp -> Z -> normalized gates -> broadcast."""
                hp = psA.tile([16, TILE], f32, tag="ps1")
                for kc in range(2):
                    nc.tensor.matmul(
                        hp[:],
                        lhsT=w1sb[:, kc, 768:784],
                        rhs=xt[:, kc, :],
                        start=(kc == 0),
                        stop=(kc == 1),
                    )
                pexp = sbg.tile([16, TILE], bf16, tag="pexp")
                nc.scalar.activation(out=pexp[:], in_=hp[:], func=Exp)
                return pexp

            def l1_chunk(xt, m):
                hp = psA.tile([128, TILE], f32, tag="ps1")
                for kc in range(2):
                    nc.tensor.matmul(
                        hp[:],
                        lhsT=w1sb[:, kc, m * 128:(m + 1) * 128],
                        rhs=xt[:, kc, :],
                        start=(kc == 0),
                        stop=(kc == 1),
                    )
                h1sb = sbb.tile([128, TILE], bf16, tag="h1sb")
                nc.scalar.activation(out=h1sb[:], in_=hp[:], func=Relu)
                return h1sb

            def gates_z(pexp):
                """Z matmul (PE part of the gate chain)."""
                zp = psA.tile([16, TILE], f32, tag="ps1")
                nc.tensor.matmul(zp[:], lhsT=o16sb[:], rhs=pexp[:],
                                 start=True, stop=True)
                return zp

            def gate_bcast(ctx):
                """Normalize gates + DRAM-roundtrip row broadcast.

                Emitted AFTER stage_b(prev) so the DVE recip/pnorm (which
                wait on this tile's Z matmul) queue behind the previous
                tile's stt chain, never in front of it.
                """
                zp, pexp = ctx.pop("zp"), ctx.pop("pexp")
                rz = sbg.tile([16, TILE], f32, tag="rz")
                nc.vector.reciprocal_approx_fast(out=rz[:], in_=zp[:])
                pnorm = sbg.tile([16, TILE], bf16, tag="pnorm")
                nc.gpsimd.tensor_mul(out=pnorm[:], in0=rz[:], in1=pexp[:])
                pscr = drp.tile([16, TILE], bf16, tag="pscr")
                nc.sync.dma_start(out=pscr[:], in_=pnorm[:])
                rowstep = pscr[:].ap[-1][0] * TILE
                pball = sbp.tile([128, 8, TILE], bf16, tag="pball")
                for half in range(2):
                    base = pscr[half:half + 1, :]
                    src = bass.AP(
                        tensor=base.tensor,
                        offset=base.offset,
                        ap=[[0, 64], [2 * rowstep, 8], [1, TILE]],
                    )
                    nc.sync.dma_start(
                        out=pball[half * 64:(half + 1) * 64, :, :],
                        in_=src,
                    )
                ctx["pball"] = pball

            def stage_a(it, xts):
                """Tile it: gates + L1 + Z (PE/Act only)."""
                if it + 1 < ntiles:
                    xts[it + 1] = load_xt(it + 1)
                xt = xts.pop(it)
                pexp = gates_head(xt)
                h1s = [l1_chunk(xt, 0)]
                zp = gates_z(pexp)
                for m in range(1, 6):
                    h1s.append(l1_chunk(xt, m))
                return dict(it=it, h1s=h1s, zp=zp, pexp=pexp)

            def stage_b1(ctx, pairs):
                """Tile it: L2 + gated stacks (PE + DVE)."""
                h1s, pball = ctx["h1s"], ctx["pball"]
                stacks = ctx.setdefault("stacks", {})
                for p in pairs:
                    h2p = psB.tile([128, TILE], f32, tag="h2")
                    nc.tensor.matmul(
                        h2p[:],
                        lhsT=w2sb[:, p * 128:(p + 1) * 128],
                        rhs=h1s[p][:],
                        start=True,
                        stop=True,
                    )
                    users = ([(p // 2, p % 2)] if p < 4
                             else [(0, p - 2), (1, p - 2)])
                    for (t, i) in users:
                        st = sbc.tile([128, TILE], bf16, tag="stack")
                        nc.vector.scalar_tensor_tensor(
                            out=st[:], in0=h2p[:], scalar=0.0,
                            in1=pball[:, t * 4 + i, :], op0=amax, op1=mult,
                        )
                        stacks[(t, i)] = st

            def stage_b2(ctx):
                """Tile it: L3 + output path."""
                it, stacks = ctx["it"], ctx["stacks"]
                lp = psC.tile([128, TILE], f32, tag="lp")
                for i in range(4):
                    for t in range(2):
                        nc.tensor.matmul(
                            lp[t * 64:(t + 1) * 64, :],
                            lhsT=w3sb[:, (t * 4 + i) * 64:(t * 4 + i + 1) * 64],
                            rhs=stacks[(t, i)][:],
                            start=(i == 0),
                            stop=(i == 3),
                            tile_position=(0, t * 64),
                            skip_group_check=True,
                        )
                outsb = sbo.tile([128, TILE], bf16, tag="outsb")
                nc.scalar.copy(out=outsb[:], in_=lp[:])
                outfin = sbo.tile([128, 4, 128], bf16, tag="outfin")
                nc.sync.dma_start_transpose(out=outfin[:], in_=outsb[:])
                nc.gpsimd.dma_start(out=OUTP[it], in_=outfin[:])

            # --- 3-deep software pipeline ---
            # iter k PE order: [L2a(k-1), L1(k)+Z(k), L2b(k-1), L3(k-2)].
            # Early L2a gives the DVE stt chain work at iteration start;
            # the gate chain (recip -> pnorm on Pool -> DRAM roundtrip)
            # completes ~half an iteration before its stts need it.
            xts = {0: load_xt(0)}
            ctxs = {}
            for k in range(ntiles + 2):
                if 0 <= k - 1 < ntiles:
                    stage_b1(ctxs[k - 1], (0, 1, 2))
                if k < ntiles:
                    ctxs[k] = stage_a(k, xts)
                    gate_bcast(ctxs[k])
                if 0 <= k - 1 < ntiles:
                    stage_b1(ctxs[k - 1], (3, 4, 5))
                if k - 2 >= 0:
                    stage_b2(ctxs.pop(k - 2))

    nc.finalize()
    return nc


def _prep_weights(Wt1, Wt2, Wt3, Ws1, Ws2, Ws3, Wg):
    """Host-side packing of weights into the layouts the kernel expects."""
    bf16 = ml_dtypes.bfloat16
    W1x = [np.asarray(Wt1[t, e], np.float32) for t in range(T) for e in range(ET)]
    W1x += [np.asarray(Ws1[e], np.float32) for e in range(ES)]
    W2x = [np.asarray(Wt2[t, e], np.float32) for t in range(T) for e in range(ET)]
    W2x += [np.asarray(Ws2[e], np.float32) for e in range(ES)]
    W3x = [np.asarray(Wt3[t, e], np.float32) for t in range(T) for e in range(ET)]
    W3x += [np.asarray(Ws3[e], np.float32) for e in range(ES)]

    # L1 weights: [256, 768] experts + [256, 16] gates -> [128, 2, 784]
    w1cat = np.concatenate(W1x + [np.asarray(Wg[0], np.float32),
                                  np.asarray(Wg[1], np.float32)], axis=1)
    assert w1cat.shape == (IN, 784)
    W1C = w1cat.reshape(2, 128, 784).transpose(1, 0, 2).astype(bf16)

    # L2 block-diagonal pairs: pair p = experts (2p, 2p+1)
    W2B = np.zeros((128, 768), np.float32)
    for p in range(6):
        W2B[0:64, p * 128:p * 128 + 64] = W2x[2 * p]
        W2B[64:128, p * 128 + 64:p * 128 + 128] = W2x[2 * p + 1]
    W2B = W2B.astype(bf16)

    # L3 stacked pairs per (task, i): stack slots (2i, 2i+1)
    W3S = np.zeros((128, 512), np.float32)
    for t in range(T):
        slot = [t * 4, t * 4 + 1, t * 4 + 2, t * 4 + 3, 8, 9, 10, 11]
        for i in range(4):
            c0 = (t * 4 + i) * 64
            W3S[0:64, c0:c0 + 64] = W3x[slot[2 * i]]
            W3S[64:128, c0:c0 + 64] = W3x[slot[2 * i + 1]]
    W3S = W3S.astype(bf16)

    # block-ones for the softmax denominator: Z[m] = sum over slots of
    # the same task as slot m
    O16h = np.zeros((16, 16), np.float32)
    O16h[0:8, 0:8] = 1.0
    O16h[8:16, 8:16] = 1.0
    return dict(W1C=W1C, W2B=W2B, W3S=W3S, O16=O16h.astype(bf16))


def make_in_maps(X, Wt1, Wt2, Wt3, Ws1, Ws2, Ws3, Wg):
    bf16 = ml_dtypes.bfloat16
    consts = _prep_weights(Wt1, Wt2, Wt3, Ws1, Ws2, Ws3, Wg)
    Xb = np.asarray(X, np.float32).astype(bf16)
    in_maps = []
    for c in range(NCORES):
        xt = np.ascontiguousarray(
            Xb[c * SHARD:(c + 1) * SHARD].T.reshape(2, 128, SHARD))
        m = {"XT": xt}
        m.update(consts)
        in_maps.append(m)
    return in_maps


def unpack_out(res):
    """[ntiles, 128, 512] bf16 per core -> [B, T, OUT] fp32."""
    outs = []
    for c in range(NCORES):
        o = np.asarray(res.results[c]["out"])
        o = o.reshape(NTILES, 128, 4, 128).transpose(0, 2, 1, 3)
        outs.append(o.reshape(SHARD, 128))
    return np.ascontiguousarray(
        np.concatenate(outs, axis=0).astype(np.float32).reshape(B, T, OUT))


def kernel(X, Wt1, bt1, Wt2, bt2, Wt3, bt3,
           Ws1, bs1, Ws2, bs2, Ws3, bs3, Wg, bg):
    from concourse.bass_utils import run_bass_kernel_spmd

    in_maps = make_in_maps(X, Wt1, Wt2, Wt3, Ws1, Ws2, Ws3, Wg)
    if "nc" not in _BUILD_CACHE:
        _BUILD_CACHE["nc"] = _build(NTILES)
    nc = _BUILD_CACHE["nc"]
    res = run_bass_kernel_spmd(nc, in_maps, list(range(NCORES)))
    return unpack_out(res)
